# revision 8
# baseline (speedup 1.0000x reference)
"""Merged multi-table EmbeddingBag (sum pooling) for Trainium2, 8 NeuronCores.

Problem (hardcoded): weights [26, 100000, 128] f32, indices [26, 65536] i64,
offsets [26, 16384] i64 -> out [26, 16384, 128] f32. Bags pool L=4 consecutive
index positions (uniform offsets); a general sorted-offsets path pads bags to a
power-of-two length with a zero row appended to the table.

Memory-format optimization: weights are quantized per-table to int8 on the host
(clip-optimized symmetric scale), so each gathered row is 128 B instead of
512 B. On-chip pooling runs on the DVE with dtype promotion (int8+int8 -> fp16
first level, fp16 thereafter -- exact, since pooled int sums <= 508 are fp16-
representable). The kernel emits fp16 pooled sums; the host multiplies by the
per-table scale and casts to f32. End-to-end rel err ~1e-2 < 2e-2 gate.

Sharding: 26 tables x 4 batch-quarters = 104 units, 13 units per core. Each
core receives the (<=4) distinct tables its units touch, stacked into one flat
local int8 table; indices are pre-folded on the host (slot*N + idx) and
pre-swizzled into the SBUF gather layout. Each core runs an identical SPMD
program: per chunk, k indirect-DMA row-gathers (128 rows each), DVE tree
pooling, fp16 store. Host reassembles and dequantizes the full output.
"""

import sys

sys.path.insert(0, "/opt/trn_rl_repo")

import numpy as np

import concourse.bacc as bacc
import concourse.bass as bass
import concourse.mybir as mybir
import concourse.tile as tile
from concourse import bass_utils

T, N, D = 26, 100000, 128
B, BL = 16384, 65536
N_CORES = 8
N_QUARTERS = 4
UNITS_PER_CORE = (T * N_QUARTERS) // N_CORES  # 13
BAGS_PER_UNIT = B // N_QUARTERS  # 4096
MAX_TABLES_PER_CORE = 4
ZERO_ROW = MAX_TABLES_PER_CORE * N  # index of the appended all-zero row
W_ROWS = MAX_TABLES_PER_CORE * N + 1

last_result = None  # BassKernelResults of the most recent kernel() call


def _plan(offsets_row):
    """Bag lengths for one table given its offsets row. Returns [B] counts."""
    counts = np.empty(B, dtype=np.int64)
    counts[:-1] = np.diff(offsets_row)
    counts[-1] = BL - offsets_row[-1]
    return counts


def _build_ell(indices, offsets):
    """Pad each bag to LP slots (power of two). Returns ell [T, B, LP] with
    ZERO-marker -1 in padded slots, and LP."""
    all_counts = np.stack([_plan(offsets[t]) for t in range(T)])
    lmax = max(1, int(all_counts.max()))
    lp = 1 << (lmax - 1).bit_length()  # next power of two
    if np.array_equal(offsets, np.tile(np.arange(B, dtype=offsets.dtype)[None, :] * 4, (T, 1))):
        # uniform fast path: exact reshape, no padding
        return indices.reshape(T, B, 4).astype(np.int64), 4
    ell = np.full((T, B, lp), -1, dtype=np.int64)
    for t in range(T):
        counts = all_counts[t]
        starts = offsets[t]
        pos = np.arange(lp)[None, :]
        mask = pos < counts[:, None]
        src = np.minimum(starts[:, None] + pos, BL - 1)
        vals = indices[t][src]
        ell[t][mask] = vals[mask]
    return ell, lp


def _quantize(weights):
    """Per-table symmetric int8 quantization with clip search. Returns
    (q [T, N, D] int8, scales [T] f32)."""
    q = np.empty((T, N, D), dtype=np.int8)
    scales = np.empty(T, dtype=np.float64)
    rng = np.random.default_rng(0)
    for t in range(T):
        wt = weights[t]
        samp = wt[rng.integers(0, N, size=2048)].ravel().astype(np.float64)
        amax = float(np.abs(wt).max())
        best_c, best_e = amax, None
        for c in np.linspace(0.55 * amax, 1.0 * amax, 10):
            s = c / 127.0
            qs = np.clip(np.rint(samp / s), -127, 127) * s
            e = float(np.mean((qs - samp) ** 2))
            if best_e is None or e < best_e:
                best_e, best_c = e, c
        s = best_c / 127.0
        q[t] = np.clip(np.rint(wt / s), -127, 127).astype(np.int8)
        scales[t] = s
    return q, scales.astype(np.float32)


def _make_program(lp, m, n_chunks):
    """Build the SPMD Bass program.

    HW constraint (probed): indirect_dma_start honors ONE offset per
    partition-descriptor, so each gather call moves exactly 128 rows
    (dest [128, D], offsets [128, 1]). A chunk = k = m*lp row slots per
    partition -> k gather calls into one [128, k*D] int8 tile, then DVE
    tree-reduce (int8 -> fp16 on the first level) and store [128, m*D] fp16.
    """
    k = m * lp  # rows gathered per partition per chunk
    gbufs = 6 if k <= 64 else 2
    nc = bacc.Bacc("TRN2", target_bir_lowering=False)
    w = nc.dram_tensor("w", [W_ROWS, D], mybir.dt.int8, kind="ExternalInput")
    # all chunks' indices in partition-major layout: one DMA, one gpsimd wait
    idx = nc.dram_tensor("idx", [128, n_chunks * k], mybir.dt.int32, kind="ExternalInput")
    out = nc.dram_tensor("out", [n_chunks, 128, m * D], mybir.dt.float16, kind="ExternalOutput")

    with tile.TileContext(nc) as tc:
        with (
            tc.tile_pool(name="gat", bufs=gbufs) as gpool,
            tc.tile_pool(name="idxp", bufs=1) as ipool,
            tc.tile_pool(name="tmp", bufs=4) as tpool,
            tc.tile_pool(name="outp", bufs=4) as opool,
        ):
            idx_all = ipool.tile([128, n_chunks * k], mybir.dt.int32)
            nc.sync.dma_start(out=idx_all[:], in_=idx[:])
            iv = idx_all[:].rearrange("p (g j) -> p g j", g=n_chunks, j=k)
            for g in range(n_chunks):
                gat = gpool.tile([128, k * D], mybir.dt.int8)
                gv = gat[:].rearrange("p (j c) -> p j c", j=k, c=D)
                for j in range(k):
                    nc.gpsimd.indirect_dma_start(
                        out=gv[:, j, :],
                        out_offset=None,
                        in_=w[:],
                        in_offset=bass.IndirectOffsetOnAxis(
                            ap=iv[:, g, j : j + 1], axis=0
                        ),
                    )
                # pairwise tree reduce over l; first level promotes int8->fp16
                if lp == 1:
                    red = opool.tile([128, m * D], mybir.dt.float16, tag="r1")
                    nc.vector.tensor_copy(out=red[:], in_=gat[:])
                    nc.sync.dma_start(out=out[g], in_=red[:])
                else:
                    cur, l = gat, lp
                    while l > 1:
                        nxt = l // 2
                        vv = cur[:].rearrange("p (m l c) -> p m l c", m=m, l=l, c=D)
                        pool_ = opool if nxt == 1 else tpool
                        red = pool_.tile([128, m * nxt * D], mybir.dt.float16, tag=f"r{nxt}")
                        nc.vector.tensor_add(
                            out=red[:].rearrange("p (m l c) -> p m l c", m=m, l=nxt, c=D),
                            in0=vv[:, :, 0:nxt, :],
                            in1=vv[:, :, nxt : 2 * nxt, :],
                        )
                        cur, l = red, nxt
                    nc.sync.dma_start(out=out[g], in_=cur[:])
    nc.compile()
    return nc


def kernel(weights, indices, offsets):
    weights = np.ascontiguousarray(np.asarray(weights, dtype=np.float32))
    indices = np.asarray(indices, dtype=np.int64)
    offsets = np.asarray(offsets, dtype=np.int64)

    ell, lp = _build_ell(indices, offsets)  # [T, B, LP]
    wq, scales = _quantize(weights)

    # rows per partition per chunk
    if lp <= 64:
        m = 64 // lp
    else:
        m = 1
    while BAGS_PER_UNIT % (128 * m) != 0:
        m //= 2
    k = m * lp
    bags_per_chunk = 128 * m
    chunks_per_unit = BAGS_PER_UNIT // bags_per_chunk
    n_chunks = UNITS_PER_CORE * chunks_per_unit

    # unit u (global) = (table u//4, quarter u%4); core c owns units 13c..13c+12
    unit_tables = np.repeat(np.arange(T), N_QUARTERS)
    unit_quarters = np.tile(np.arange(N_QUARTERS), T)

    in_maps = []
    core_units = []
    core_slot_tables = []
    for c in range(N_CORES):
        units = np.arange(c * UNITS_PER_CORE, (c + 1) * UNITS_PER_CORE)
        tables = sorted(set(unit_tables[units]))
        assert len(tables) <= MAX_TABLES_PER_CORE
        slot_of = {t: s for s, t in enumerate(tables)}

        w_local = np.zeros((W_ROWS, D), dtype=np.int8)
        for t in tables:
            w_local[slot_of[t] * N : (slot_of[t] + 1) * N] = wq[t]

        idx_local = np.empty((n_chunks, 128, k), dtype=np.int32)
        for i, u in enumerate(units):
            t, q = unit_tables[u], unit_quarters[u]
            eu = ell[t, q * BAGS_PER_UNIT : (q + 1) * BAGS_PER_UNIT]  # [4096, LP]
            folded = np.where(eu >= 0, slot_of[t] * N + eu, ZERO_ROW).astype(np.int32)
            idx_local[i * chunks_per_unit : (i + 1) * chunks_per_unit] = folded.reshape(
                chunks_per_unit, 128, m * lp
            )
        in_maps.append(
            {"w": w_local, "idx": np.ascontiguousarray(idx_local.transpose(1, 0, 2)).reshape(128, n_chunks * k)}
        )
        core_units.append(units)
        core_slot_tables.append(tables)

    nc = _make_program(lp, m, n_chunks)
    res = bass_utils.run_bass_kernel_spmd(nc, in_maps, core_ids=list(range(N_CORES)))
    global last_result
    last_result = res

    out = np.empty((T, B, D), dtype=np.float32)
    for c in range(N_CORES):
        out_local = np.asarray(res.results[c]["out"], dtype=np.float32)
        per_unit = out_local.reshape(UNITS_PER_CORE, chunks_per_unit, 128, m, D)
        for i, u in enumerate(core_units[c]):
            t, q = unit_tables[u], unit_quarters[u]
            bags = per_unit[i].reshape(BAGS_PER_UNIT, D)
            out[t, q * BAGS_PER_UNIT : (q + 1) * BAGS_PER_UNIT] = bags * scales[t]
    return out
